# revision 1
# baseline (speedup 1.0000x reference)
"""Trainium2 Bass kernel for nn_Decoder_46634754900483.

Data-parallel over batch: 8 cores x 2048 rows.  All big per-step tensors
live in a "folded-T" layout (128 partitions = (n%2)*64 + d, free columns =
(bt, k, b%128) with n = 2k + parity), produced by 128-column DMA xbar
transposes.  Per step:
  arg   = pre + h @ Wc1bot        (PE: identity-matmul + h-matmul into PSUM)
  x1    = tanh(arg + bc1)         (ACT, bias per partition)
  compab= x1 . Wc2                (PE: ldw x1-slice (128,128), N=2 matmul)
  e     = exp(compab)             (ACT), s = sum_n e, rec = 1/s (DVE)
  e_rep = broadcast e over d      (PE transpose -> DRAM -> broadcast DMA)
  prod  = int_Tf * e_rep          (DVE)
  N     = sum_n prod              (PE: [I;I] fold-matmul accumulating over k)
  weighted = N * rec_rep          (DVE, psum input)
  emb/GRU/y                       (PE matmuls + ACT/DVE elementwise)

PSUM (8 banks) is managed manually as one (128, 4096) f32 tensor with
phase-local slice views; Tile's shadow-memory dep tracking covers it.
"""
import sys
import os

sys.path.insert(0, "/opt/trn_rl_repo")

import numpy as np
import ml_dtypes

BF = ml_dtypes.bfloat16

B = 16384
NC = 8
BL = B // NC            # 2048
ND = 20
NK = ND // 2            # 10
D = 64                  # d_g == d_h
DE = 16
T = 12
NBT = BL // 128         # 16
COLS = NBT * NK * 128   # 20480

_BUILT = {}



def _patch_tile_for_walrus():
    """This container's walrus build rejects instructions carrying more than
    one sync wait.  Split extra waits onto same-engine NOPs (engine streams
    are in-order, so semantics are preserved)."""
    import concourse.tile as tile
    from concourse.tile import ScopedClock
    import concourse.mybir as mybir

    if getattr(tile.TileContext, "_walrus_wait_patched", False):
        return

    _orig_add_instruction = tile.TileContext._add_instruction

    def _add_instruction_split(self, inst):
        si = inst.sync_info
        if si and si.on_wait and len(si.on_wait) > 1:
            engine = self.nc.engines[inst.engine]
            waits = list(si.on_wait)
            si.on_wait = waits[-1:]
            for w in waits[:-1]:
                nop = engine.nop(nofuse=True)
                nsi = nop.ins.sync_info
                if nsi is None:
                    nop.ins.sync_info = mybir.SyncInfo(on_wait=[w],
                                                       on_update=[])
                else:
                    nsi.on_wait = [w]
        _orig_add_instruction(self, inst)

    def _drain_and_barrier_split(self, tick_clock, wait_clock):
        nc = self.nc
        drain_inst = nc.sync.drain()
        wait_clock.add_sem_waits(
            drain_inst.ins, ScopedClock({None: tick_clock.global_clock}))
        si = drain_inst.ins.sync_info
        waits = list(si.on_wait) if si and si.on_wait else []
        if len(waits) > 1:
            si.on_wait = waits[:1]
            for w in waits[1:]:
                nop = nc.sync.nop(nofuse=True)
                nsi = nop.ins.sync_info
                if nsi is None:
                    nop.ins.sync_info = mybir.SyncInfo(on_wait=[w],
                                                       on_update=[])
                else:
                    nsi.on_wait = [w]
        nc.all_engine_barrier()
        assert self.sems is not None
        popped = nc._tile_sem_poison_stack.pop()
        assert popped is self._sem_poison
        nc.clear_and_free_semaphores(list(self.sems.allocated().values()))
        nc.all_engine_barrier()

    tile.TileContext._add_instruction = _add_instruction_split
    tile.TileContext._drain_and_barrier = _drain_and_barrier_split
    tile.TileContext._walrus_wait_patched = True

def _build():
    _patch_tile_for_walrus()
    from contextlib import ExitStack
    import concourse.bass as bass
    import concourse.tile as tile
    from concourse import mybir

    F32, BF16 = mybir.dt.float32, mybir.dt.bfloat16
    AF = mybir.ActivationFunctionType
    ALU = mybir.AluOpType
    AX = mybir.AxisListType

    nc = bass.Bass("TRN2", target_bir_lowering=False)

    def param(name, shape, dt=BF16):
        return nc.declare_dram_parameter(name, list(shape), dt, isOutput=False)

    int_d = param("int_nat", [128, NBT * ND * D])          # bf16, (p,(bt,n,d))
    h0_d = param("h0T", [D, BL])                           # bf16
    y0_d = param("y0T", [3, BL])                           # bf16 [y;ones]
    identf_d = param("identf", [128, 128], F32)
    wtop_d = param("wtop_dup", [128, D])                   # Wc1top dup'd halves
    wbot_d = param("wbot_wide", [D, 128])                  # [Wc1bot|Wc1bot]
    identb_d = param("identb", [128, 128])                 # bf16 identity
    bc1_d = param("bc1_fold", [128, 1], F32)               # [bc1;bc1]
    wc2_d = param("wc2_pair", [128, 2])                    # [[Wc2|0],[0|Wc2]]
    foldm_d = param("foldmat", [128, D])                   # [I64;I64] bf16
    weX_d = param("weX", [67, DE])                         # [We(66); be]
    wih_rz_d = param("wih_rz", [DE, 128])                  # r|z gates
    whh_rz_d = param("whh_rz", [D, 128])
    wih_n_d = param("wih_n", [DE, D])
    whh_n_d = param("whh_n", [D, D])
    brz_d = param("brz_half", [128, 1], F32)               # 0.5*(bih+bhh) r|z
    bihn_d = param("bih_n", [D, 1], F32)
    bhhn_d = param("bhh_n", [D, 1], F32)
    wo_d = param("wo", [D, 2])
    bo_d = param("bo", [2, 1], F32)

    out_d = nc.declare_dram_parameter("ys", [T, BL, 2], F32, isOutput=True)

    eT_dram = nc.dram_tensor("eT_scratch", [ND, BL], BF16)
    recT_dram = nc.dram_tensor("recT_scratch", [NBT, 128], BF16)

    with ExitStack() as octx:
        PSraw = octx.enter_context(nc.psum_tensor([128, 4096], F32))
        with tile.TileContext(nc) as tc, ExitStack() as ctx:
            PS = PSraw
            wp = ctx.enter_context(tc.tile_pool(name="weights", bufs=1))
            big = ctx.enter_context(tc.tile_pool(name="big", bufs=1))
            bigx = ctx.enter_context(tc.tile_pool(name="bigx", bufs=2))
            bige = ctx.enter_context(tc.tile_pool(name="bige", bufs=1))
            sm = ctx.enter_context(tc.tile_pool(name="small", bufs=1))

            # ---------------- load weights / state ----------------
            identf = wp.tile([128, 128], F32)
            nc.sync.dma_start(identf[:], identf_d[:])
            identb = wp.tile([128, 128], BF16)
            nc.sync.dma_start(identb[:], identb_d[:])
            wtop = wp.tile([128, D], BF16)
            nc.sync.dma_start(wtop[:], wtop_d[:])
            wbot = wp.tile([D, 128], BF16)
            nc.sync.dma_start(wbot[:], wbot_d[:])
            bc1f = wp.tile([128, 1], F32)
            nc.sync.dma_start(bc1f[:], bc1_d[:])
            wc2p = wp.tile([128, 2], BF16)
            nc.sync.dma_start(wc2p[:], wc2_d[:])
            foldm = wp.tile([128, D], BF16)
            nc.sync.dma_start(foldm[:], foldm_d[:])
            weX = wp.tile([67, DE], BF16)
            nc.sync.dma_start(weX[:], weX_d[:])
            wih_rz = wp.tile([DE, 128], BF16)
            nc.sync.dma_start(wih_rz[:], wih_rz_d[:])
            whh_rz = wp.tile([D, 128], BF16)
            nc.sync.dma_start(whh_rz[:], whh_rz_d[:])
            wih_n = wp.tile([DE, D], BF16)
            nc.sync.dma_start(wih_n[:], wih_n_d[:])
            whh_n = wp.tile([D, D], BF16)
            nc.sync.dma_start(whh_n[:], whh_n_d[:])
            brz = wp.tile([128, 1], F32)
            nc.sync.dma_start(brz[:], brz_d[:])
            bihn = wp.tile([D, 1], F32)
            nc.sync.dma_start(bihn[:], bihn_d[:])
            bhhn = wp.tile([D, 1], F32)
            nc.sync.dma_start(bhhn[:], bhhn_d[:])
            wo = wp.tile([D, 2], BF16)
            nc.sync.dma_start(wo[:], wo_d[:])
            bo = wp.tile([2, 1], F32)
            nc.sync.dma_start(bo[:], bo_d[:])

            hTd = wp.tile([128, BL], BF16)      # h duplicated on both halves
            nc.sync.dma_start(hTd[0:D, :], h0_d[:])
            nc.sync.dma_start(hTd[D:128, :], h0_d[:])
            embrhs = wp.tile([67, BL], BF16)    # [weighted; y; ones]
            nc.sync.dma_start(embrhs[64:67, :], y0_d[:])

            # ---------------- int_Tf via xbar transposes ----------------
            int_Tf = big.tile([128, NBT, NK, 128], BF16)
            for bt in range(NBT):
                for j in range(NK):
                    off = bt * ND * D + j * 128
                    nc.sync.dma_start_transpose(
                        int_Tf[:, bt, j, :], int_d[:, off:off + 128])
            int_flat = int_Tf[:].rearrange("p a b c -> p (a b c)")

            # ---------------- pre = int @ Wc1top  (folded-T) ----------------
            pre = big.tile([128, NBT, NK, 128], BF16)
            pre_flat = pre[:].rearrange("p a b c -> p (a b c)")
            NBLK = NBT * NK  # 160 column blocks of 128
            for g in range(NBLK // 4):
                ps = PS[:, (g % 2) * 512:(g % 2) * 512 + 512]
                c0 = g * 512
                for half in range(2):
                    rows = slice(half * D, (half + 1) * D)
                    nc.tensor.matmul(
                        ps[rows, :], wtop[rows, :], int_flat[rows, c0:c0 + 512],
                        start=True, stop=True)
                nc.vector.tensor_copy(pre_flat[:, c0:c0 + 512], ps[:])

            # ---------------- time loop ----------------
            for t in range(T):
                # --- arg/x1: per bt, psum (128,1280) = pre + h-part
                x1 = bigx.tile([128, NBT, NK, 128], BF16, tag="bigx")
                cp = PS[:, 3072:3072 + NBT * ND]  # (128, 320), bank 6
                for bt in range(NBT):
                    base = 0 if (bt % 2 == 0) else 1536
                    ps = PS[:, base:base + NK * 128]
                    c0 = bt * NK * 128
                    # identity-matmul: psum = pre (both halves at once)
                    for s0 in range(0, NK * 128, 512):
                        w = min(512, NK * 128 - s0)
                        nc.tensor.matmul(ps[:, s0:s0 + w], identb[:],
                                         pre_flat[:, c0 + s0:c0 + s0 + w],
                                         start=True, stop=False)
                    # h-part: lhsT = [Wc1bot|Wc1bot] (64,128), rhs = hT bcast
                    for s0 in range(0, NK * 128, 512):
                        w = min(512, NK * 128 - s0)
                        nk = w // 128
                        hrep = hTd[0:D, bt * 128:(bt + 1) * 128].unsqueeze(1)
                        hrep = hrep.broadcast_to([D, nk, 128])
                        nc.tensor.matmul(
                            ps[:, s0:s0 + w].rearrange(
                                "p (k b) -> p k b", k=nk),
                            wbot[:], hrep, start=False, stop=True)
                    nc.scalar.activation(
                        x1[:, bt, :, :],
                        ps[:].rearrange("p (k b) -> p k b", k=NK),
                        AF.Tanh, bias=bc1f[:], scale=1.0)
                    # compab for this bt: ldw x1 slice, N=2 matmul
                    for k in range(NK):
                        col = bt * ND + 2 * k
                        nc.tensor.matmul(cp[:, col:col + 2],
                                         x1[:, bt, k, :], wc2p[:],
                                         start=True, stop=True)

                e_nat = sm.tile([128, NBT * ND], F32, tag="enat")
                nc.scalar.activation(e_nat[:], cp[:], AF.Exp)
                s_nat = sm.tile([128, NBT], F32, tag="snat")
                nc.vector.tensor_reduce(
                    s_nat[:], e_nat[:].rearrange("p (bt n) -> p bt n", n=ND),
                    axis=AX.X, op=ALU.add)
                rec = sm.tile([128, NBT], F32, tag="rec")
                nc.vector.reciprocal(rec[:], s_nat[:])

                # --- eT via PE transposes (bf16): psum bank 7
                eT = sm.tile([ND, BL], BF16, tag="eT")
                pse = PS[0:ND, 3584:4096]
                for g in range(4):
                    for q in range(4):
                        bt = g * 4 + q
                        nc.tensor.transpose(
                            pse[:, q * 128:(q + 1) * 128],
                            e_nat[:, bt * ND:(bt + 1) * ND], identf[:])
                    nc.vector.tensor_copy(eT[:, g * 512:(g + 1) * 512],
                                          pse[:])
                nc.sync.dma_start(eT_dram[:], eT[:])

                # --- rec_T -> dram -> broadcast (64, NBT, 128) f32
                psr = PS[0:NBT, 3584:3584 + 128]
                nc.tensor.transpose(psr[:], rec[:], identf[:])
                recT_sb = sm.tile([NBT, 128], BF16, tag="recTsb")
                nc.vector.tensor_copy(recT_sb[:], psr[:])
                nc.sync.dma_start(recT_dram[:], recT_sb[:])
                rec_rep = sm.tile([D, NBT, 128], BF16, tag="recrep")
                nc.sync.dma_start(
                    rec_rep[:],
                    recT_dram[:].unsqueeze(0).broadcast_to([D, NBT, 128]))

                # --- e_rep broadcast from DRAM
                erep = bigx.tile([128, NBT, NK, 128], BF16, tag="bigx")
                for half in range(2):
                    for k in range(NK):
                        srcrow = eT_dram[2 * k + half:2 * k + half + 1, :]
                        src3 = srcrow.rearrange(
                            "one (bt b) -> one bt b", bt=NBT)
                        src3 = src3.broadcast_to([D, NBT, 128])
                        nc.sync.dma_start(
                            erep[half * D:(half + 1) * D, :, k, :], src3)

                # --- product & fold-reduce into PS[0:64, 0:2048]
                # product in place over erep (read-before-write per element),
                # split by bt-quarter so fold matmuls overlap the multiplies
                prod = erep
                psN = PS[0:D, 0:2048]
                QW = NBT // 4 * NK * 128  # cols per quarter
                for q in range(4):
                    bts = slice(q * 4, (q + 1) * 4)
                    nc.vector.tensor_tensor(
                        prod[:, bts, :, :].rearrange("p a b c -> p (a b c)"),
                        int_Tf[:, bts, :, :].rearrange("p a b c -> p (a b c)"),
                        erep[:, bts, :, :].rearrange("p a b c -> p (a b c)"),
                        ALU.mult)
                    for k in range(NK):
                        nc.tensor.matmul(
                            psN[:, q * 512:(q + 1) * 512],
                            foldm[:], prod[:, bts, k, :],
                            start=(k == 0), stop=(k == NK - 1))

                # --- weighted/emb/embT per b-quarter: pipelines behind
                # the fold matmuls (quarter q ready after its k-loop)
                psE = PS[0:DE, 2048:4096]
                embT = sm.tile([DE, BL], BF16, tag="eT")
                for q in range(4):
                    qs = slice(q * 512, (q + 1) * 512)
                    nc.vector.tensor_tensor(
                        embrhs[0:D, qs].rearrange(
                            "d (bt b) -> d bt b", bt=4),
                        psN[:, qs].rearrange("d (bt b) -> d bt b", bt=4),
                        rec_rep[:, q * 4:(q + 1) * 4, :], ALU.mult)
                    nc.tensor.matmul(psE[:, qs], weX[:], embrhs[:, qs],
                                     start=True, stop=True)
                    nc.vector.tensor_copy(embT[:, qs], psE[:, qs])

                # --- GRU gates in two b-halves, base-0 psums (64, 1024)
                HB = BL // 2
                hnew = sm.tile([D, BL], BF16, tag="hnew")
                ps_r = PS[0:D, 0:1024]
                ps_z = PS[0:D, 1024:2048]
                ps_i = PS[0:D, 2048:3072]
                ps_h = PS[0:D, 3072:4096]
                for hb in range(2):
                    cs = slice(hb * HB, (hb + 1) * HB)
                    for q in range(2):
                        qs = slice(hb * HB + q * 512, hb * HB + (q + 1) * 512)
                        qo = slice(q * 512, (q + 1) * 512)
                        nc.tensor.matmul(ps_r[:, qo], wih_rz[:, 0:D],
                                         embT[:, qs], start=True, stop=False)
                        nc.tensor.matmul(ps_r[:, qo], whh_rz[:, 0:D],
                                         hTd[0:D, qs], start=False, stop=True)
                        nc.tensor.matmul(ps_z[:, qo], wih_rz[:, D:128],
                                         embT[:, qs], start=True, stop=False)
                        nc.tensor.matmul(ps_z[:, qo], whh_rz[:, D:128],
                                         hTd[0:D, qs], start=False, stop=True)
                        nc.tensor.matmul(ps_i[:, qo], wih_n[:],
                                         embT[:, qs], start=True, stop=True)
                        nc.tensor.matmul(ps_h[:, qo], whh_n[:],
                                         hTd[0:D, qs], start=True, stop=True)
                    # r,z = sigmoid via tanh trick
                    tr = sm.tile([D, HB], BF16, tag="gA")
                    nc.scalar.activation(tr[:], ps_r[:], AF.Tanh,
                                         bias=brz[0:D, :], scale=0.5)
                    r_sb = sm.tile([D, HB], BF16, tag="gB")
                    nc.vector.tensor_scalar(r_sb[:], tr[:], 0.5, 0.5,
                                            ALU.mult, ALU.add)
                    tz = sm.tile([D, HB], BF16, tag="gA")
                    nc.scalar.activation(tz[:], ps_z[:], AF.Tanh,
                                         bias=brz[D:128, :], scale=0.5)
                    z_sb = sm.tile([D, HB], BF16, tag="z_sb")
                    nc.vector.tensor_scalar(z_sb[:], tz[:], 0.5, 0.5,
                                            ALU.mult, ALU.add)
                    # rhn = (hn + bhh_n) * r ; narg = (inn + bih_n) + rhn
                    rhn = sm.tile([D, HB], BF16, tag="rhn")
                    nc.vector.scalar_tensor_tensor(
                        rhn[:], ps_h[:], bhhn[:], r_sb[:], ALU.add, ALU.mult)
                    narg = sm.tile([D, HB], BF16, tag="narg")
                    nc.vector.scalar_tensor_tensor(
                        narg[:], ps_i[:], bihn[:], rhn[:], ALU.add, ALU.add)
                    n_sb = sm.tile([D, HB], BF16, tag="n_sb")
                    nc.scalar.activation(n_sb[:], narg[:], AF.Tanh)
                    # h_new = n + z*(h-n)
                    hmn = sm.tile([D, HB], BF16, tag="gB")
                    nc.vector.tensor_tensor(hmn[:], hTd[0:D, cs], n_sb[:],
                                            ALU.subtract)
                    zt = sm.tile([D, HB], BF16, tag="gA")
                    nc.vector.tensor_tensor(zt[:], hmn[:], z_sb[:], ALU.mult)
                    nc.vector.tensor_tensor(hnew[:, cs], zt[:], n_sb[:],
                                            ALU.add)

                # write back h (both halves) via DMA
                nc.sync.dma_start(hTd[0:D, :], hnew[:])
                nc.sync.dma_start(hTd[D:128, :], hnew[:])

                # --- y = h_new @ Wo + bo : psY = PS[0:2, 0:2048]
                psY = PS[0:2, 0:2048]
                for q in range(4):
                    nc.tensor.matmul(psY[:, q * 512:(q + 1) * 512], wo[:],
                                     hnew[:, q * 512:(q + 1) * 512],
                                     start=True, stop=True)
                yTf = sm.tile([2, BL], F32, tag="yTf")
                nc.scalar.activation(yTf[:], psY[:], AF.Identity, bias=bo[:])
                nc.gpsimd.dma_start(embrhs[64:66, :], yTf[:])
                nc.sync.dma_start(out_d[t, :, :].rearrange("b c -> c b"),
                                  yTf[:])

    return nc


def _prep_inputs(last_x_rel, zo, intermediate, We, be, Wih, Whh, bih, bhh,
                 Wc1, bc1, Wc2, bc2, Wo, bo):
    """Shard + pack host-side.  Returns list of per-core input dicts."""
    lx = np.asarray(last_x_rel, np.float32)
    zo = np.asarray(zo, np.float32)
    inter = np.asarray(intermediate, np.float32)
    We = np.asarray(We, np.float32)
    be = np.asarray(be, np.float32)
    Wih = np.asarray(Wih, np.float32)
    Whh = np.asarray(Whh, np.float32)
    bih = np.asarray(bih, np.float32)
    bhh = np.asarray(bhh, np.float32)
    Wc1 = np.asarray(Wc1, np.float32)
    bc1 = np.asarray(bc1, np.float32)
    Wc2 = np.asarray(Wc2, np.float32)
    bc2 = np.asarray(bc2, np.float32)
    Wo = np.asarray(Wo, np.float32)
    bo = np.asarray(bo, np.float32)

    identf = np.eye(128, dtype=np.float32)
    identb = np.eye(128, dtype=np.float32).astype(BF)
    wtop_dup = np.concatenate([Wc1[0:D], Wc1[0:D]], 0).astype(BF)
    wbot_wide = np.concatenate([Wc1[D:2 * D], Wc1[D:2 * D]], 1).astype(BF)
    bc1_fold = np.concatenate([bc1, bc1])[:, None].astype(np.float32)
    wc2_pair = np.zeros((128, 2), np.float32)
    wc2_pair[0:D, 0] = Wc2[:, 0]
    wc2_pair[D:128, 1] = Wc2[:, 0]
    wc2_pair = wc2_pair.astype(BF)
    foldmat = np.concatenate([np.eye(D), np.eye(D)], 0).astype(BF)
    weX = np.concatenate([We, be[None, :]], 0).astype(BF)   # (67, 16)
    wih_rz = Wih[:, 0:128].astype(BF)
    whh_rz = Whh[:, 0:128].astype(BF)
    wih_n = Wih[:, 128:192].astype(BF)
    whh_n = Whh[:, 128:192].astype(BF)
    brz_half = (0.5 * (bih[0:128] + bhh[0:128]))[:, None].astype(np.float32)
    bih_n = bih[128:192][:, None].astype(np.float32)
    bhh_n = bhh[128:192][:, None].astype(np.float32)
    wo_b = Wo.astype(BF)
    bo_c = bo[:, None].astype(np.float32)

    iv = inter.reshape(ND, NC, NBT, 128, D)
    zv = zo[-1]  # (B, D)
    in_maps = []
    for c in range(NC):
        int_nat = np.ascontiguousarray(
            iv[:, c].transpose(2, 1, 0, 3).reshape(128, NBT * ND * D)
        ).astype(BF)
        h0T = np.ascontiguousarray(zv[c * BL:(c + 1) * BL].T).astype(BF)
        y0T = np.concatenate([lx[c * BL:(c + 1) * BL].T,
                              np.ones((1, BL), np.float32)], 0).astype(BF)
        in_maps.append({
            "int_nat": int_nat, "h0T": h0T, "y0T": y0T,
            "identf": identf, "identb": identb,
            "wtop_dup": wtop_dup, "wbot_wide": wbot_wide,
            "bc1_fold": bc1_fold, "wc2_pair": wc2_pair, "foldmat": foldmat,
            "weX": weX, "wih_rz": wih_rz, "whh_rz": whh_rz,
            "wih_n": wih_n, "whh_n": whh_n, "brz_half": brz_half,
            "bih_n": bih_n, "bhh_n": bhh_n, "wo": wo_b, "bo": bo_c,
        })
    return in_maps


def kernel(pred_len, last_x_rel, zo, zg, intermediate, We, be, Wih, Whh,
           bih, bhh, Wc1, bc1, Wc2, bc2, Wo, bo, _trace=False):
    assert int(pred_len) == T, f"kernel compiled for pred_len={T}"
    from concourse.bass_utils import run_bass_kernel_spmd

    if "nc" not in _BUILT:
        _BUILT["nc"] = _build()
    nc = _BUILT["nc"]

    in_maps = _prep_inputs(last_x_rel, zo, intermediate, We, be, Wih, Whh,
                           bih, bhh, Wc1, bc1, Wc2, bc2, Wo, bo)
    res = run_bass_kernel_spmd(nc, in_maps, list(range(NC)), trace=_trace)
    ys = np.concatenate([res.results[c]["ys"] for c in range(NC)], axis=1)
    if _trace:
        _BUILT["last_result"] = res
    return ys.astype(np.float32)



# revision 6
# speedup vs baseline: 1.4500x; 1.4500x over previous
"""Trainium2 Bass kernel for nn_Decoder_46634754900483.

Data-parallel over batch: 8 cores x 2048 rows.  Two SBUF-resident copies of
`intermediate` per core:
  int_Tf  "folded-T": partitions p=(n%2)*64+d, free (bt, k, b%128), n=2k+par.
          Feeds the x1 path (pre = int @ Wc1top precomputed once).
  intB    "b-major":  partitions b%128, free (bt, d, n).
          Feeds the attention-weighted sum.

Per step (per bt block of 128 batch rows):
  arg   = pre + (Wc1bot^T h) broadcast over k     (PE identity+h matmuls into
          PSUM for bts in PE_ARG set, DVE tensor_tensor with stride-0 k-
          broadcast of hW for the rest)
  x1    = tanh(arg + bc1)                         (ACT, bias per partition)
  cp    = x1 . Wc2  (PE: ldw x1-slice, N=2 matmul -> (128b, (bt,n)) PSUM)
  e     = exp(cp)   (ACT, per bt-quarter), s = sum_n e, alpha = e/s  (DVE)
  prod  = intB * alpha  (DVE stride-0 d-broadcast, all-SBUF bf16 2x mode)
  N     = sum_n prod    (PE: 10 transpose-matmuls per bt, lhsT=prod n-pair
          slice, rhs=identity, accumulated in a (128,128) PSUM slot; the two
          64-row halves hold even/odd n sums)
  weighted = half0+half1 (DVE psum pair-add -> embrhs rows 0:64)
  emb/GRU/y                (PE matmuls + ACT/DVE elementwise as baseline)

No per-step DRAM traffic except the (2, 2048) bf16 y output write.
"""
import sys
import os

sys.path.insert(0, "/opt/trn_rl_repo")

import numpy as np
import ml_dtypes

BF = ml_dtypes.bfloat16

B = 16384
NC = 8
BL = B // NC            # 2048
ND = 20
NK = ND // 2            # 10
D = 64                  # d_g == d_h
DE = 16
T = 12
NBT = BL // 128         # 16
NQ = 4                  # bt quarters

# bt indices whose arg-add runs on PE (identity+h streams); rest on DVE
PE_ARG = set(range(16))

_BUILT = {}


def _patch_tile_for_walrus():
    """This container's walrus build rejects instructions carrying more than
    one sync wait.  Split extra waits onto same-engine NOPs (engine streams
    are in-order, so semantics are preserved)."""
    import concourse.tile as tile
    from concourse.tile import ScopedClock
    import concourse.mybir as mybir

    if getattr(tile.TileContext, "_walrus_wait_patched", False):
        return

    _orig_add_instruction = tile.TileContext._add_instruction

    def _add_instruction_split(self, inst):
        si = inst.sync_info
        if si and si.on_wait and len(si.on_wait) > 1:
            engine = self.nc.engines[inst.engine]
            waits = list(si.on_wait)
            si.on_wait = waits[-1:]
            for w in waits[:-1]:
                nop = engine.nop(nofuse=True)
                nsi = nop.ins.sync_info
                if nsi is None:
                    nop.ins.sync_info = mybir.SyncInfo(on_wait=[w],
                                                       on_update=[])
                else:
                    nsi.on_wait = [w]
        _orig_add_instruction(self, inst)

    def _drain_and_barrier_split(self, tick_clock, wait_clock):
        nc = self.nc
        drain_inst = nc.sync.drain()
        wait_clock.add_sem_waits(
            drain_inst.ins, ScopedClock({None: tick_clock.global_clock}))
        si = drain_inst.ins.sync_info
        waits = list(si.on_wait) if si and si.on_wait else []
        if len(waits) > 1:
            si.on_wait = waits[:1]
            for w in waits[1:]:
                nop = nc.sync.nop(nofuse=True)
                nsi = nop.ins.sync_info
                if nsi is None:
                    nop.ins.sync_info = mybir.SyncInfo(on_wait=[w],
                                                       on_update=[])
                else:
                    nsi.on_wait = [w]
        nc.all_engine_barrier()
        assert self.sems is not None
        popped = nc._tile_sem_poison_stack.pop()
        assert popped is self._sem_poison
        nc.clear_and_free_semaphores(list(self.sems.allocated().values()))
        nc.all_engine_barrier()

    tile.TileContext._add_instruction = _add_instruction_split
    tile.TileContext._drain_and_barrier = _drain_and_barrier_split
    tile.TileContext._walrus_wait_patched = True


def _build():
    _patch_tile_for_walrus()
    from contextlib import ExitStack
    import concourse.bass as bass
    import concourse.tile as tile
    from concourse import mybir

    F32, BF16 = mybir.dt.float32, mybir.dt.bfloat16
    AF = mybir.ActivationFunctionType
    ALU = mybir.AluOpType
    AX = mybir.AxisListType

    nc = bass.Bass("TRN2", target_bir_lowering=False)

    def param(name, shape, dt=BF16):
        return nc.declare_dram_parameter(name, list(shape), dt, isOutput=False)

    int_d = param("int_nat", [128, NBT * ND * D])          # bf16, (p,(bt,n,d))
    intB_d = param("intB_nat", [128, NBT * D * ND])        # bf16, (b,(bt,d,n))
    h0_d = param("h0T", [D, BL])                           # bf16
    y0_d = param("y0T", [3, BL])                           # bf16 [y;ones]
    identb_d = param("identb", [128, 128])                 # bf16 identity
    wtop_d = param("wtop_dup", [128, D])                   # Wc1top dup'd halves
    wbot_d = param("wbot_wide", [D, 128])                  # [Wc1bot|Wc1bot]
    bc1_d = param("bc1_fold", [128, 1], F32)               # [bc1;bc1]
    wc2_d = param("wc2_pair", [128, 2])                    # [[Wc2|0],[0|Wc2]]
    weX_d = param("weX", [67, DE])                         # [We(66); be]
    wih_rz_d = param("wih_rz", [DE, 128])                  # r|z gates
    whh_rz_d = param("whh_rz", [D, 128])
    wih_n_d = param("wih_n", [DE, D])
    whh_n_d = param("whh_n", [D, D])
    brz_d = param("brz_half", [128, 1], F32)               # 0.5*(bih+bhh) r|z
    bihn_d = param("bih_n", [D, 1], F32)
    bhhn_d = param("bhh_n", [D, 1], F32)
    wo65_d = param("wo65", [D + 1, 2])                     # [Wo; bo]

    out_d = nc.declare_dram_parameter("ys", [T, BL, 2], BF16, isOutput=True)

    with ExitStack() as octx:
        PSraw = octx.enter_context(nc.psum_tensor([128, 4096], F32))
        with tile.TileContext(nc) as tc, ExitStack() as ctx:
            PS = PSraw
            wp = ctx.enter_context(tc.tile_pool(name="weights", bufs=1))
            big = ctx.enter_context(tc.tile_pool(name="big", bufs=1))
            x1p = ctx.enter_context(tc.tile_pool(name="x1p", bufs=4))
            prp = ctx.enter_context(tc.tile_pool(name="prp", bufs=4))
            sm = ctx.enter_context(tc.tile_pool(name="small", bufs=1))
            smq = ctx.enter_context(tc.tile_pool(name="smq", bufs=4))
            gp = ctx.enter_context(tc.tile_pool(name="gru", bufs=2))

            # ---------------- load weights / state ----------------
            identb = wp.tile([128, 128], BF16)
            nc.sync.dma_start(identb[:], identb_d[:])
            wtop = wp.tile([128, D], BF16)
            nc.sync.dma_start(wtop[:], wtop_d[:])
            wbot = wp.tile([D, 128], BF16)
            nc.sync.dma_start(wbot[:], wbot_d[:])
            bc1f = wp.tile([128, 1], F32)
            nc.sync.dma_start(bc1f[:], bc1_d[:])
            wc2p = wp.tile([128, 2], BF16)
            nc.sync.dma_start(wc2p[:], wc2_d[:])
            weX = wp.tile([67, DE], BF16)
            nc.sync.dma_start(weX[:], weX_d[:])
            wih_rz = wp.tile([DE, 128], BF16)
            nc.sync.dma_start(wih_rz[:], wih_rz_d[:])
            whh_rz = wp.tile([D, 128], BF16)
            nc.sync.dma_start(whh_rz[:], whh_rz_d[:])
            wih_n = wp.tile([DE, D], BF16)
            nc.sync.dma_start(wih_n[:], wih_n_d[:])
            whh_n = wp.tile([D, D], BF16)
            nc.sync.dma_start(whh_n[:], whh_n_d[:])
            brz = wp.tile([128, 1], F32)
            nc.sync.dma_start(brz[:], brz_d[:])
            bihn = wp.tile([D, 1], F32)
            nc.sync.dma_start(bihn[:], bihn_d[:])
            bhhn = wp.tile([D, 1], F32)
            nc.sync.dma_start(bhhn[:], bhhn_d[:])
            wo65 = wp.tile([D + 1, 2], BF16)
            nc.sync.dma_start(wo65[:], wo65_d[:])

            # h ping-pong tiles, row 64 = ones (for the bo bias row)
            hA = wp.tile([D + 1, BL], BF16)
            hB = wp.tile([D + 1, BL], BF16)
            nc.sync.dma_start(hA[0:D, :], h0_d[:])
            nc.vector.memset(hA[D:D + 1, :], 1.0)
            nc.vector.memset(hB[D:D + 1, :], 1.0)
            htiles = [hA, hB]

            embrhs = wp.tile([67, BL], BF16)    # [weighted; y; ones]
            nc.sync.dma_start(embrhs[64:67, :], y0_d[:])

            # b-major int copy
            intB = big.tile([128, NBT, D, ND], BF16)
            nc.sync.dma_start(
                intB[:].rearrange("p a b c -> p (a b c)"), intB_d[:])

            # ---------------- int_Tf via xbar transposes ----------------
            int_Tf = big.tile([128, NBT, NK, 128], BF16)
            for bt in range(NBT):
                for j in range(NK):
                    off = bt * ND * D + j * 128
                    nc.sync.dma_start_transpose(
                        int_Tf[:, bt, j, :], int_d[:, off:off + 128])
            int_flat = int_Tf[:].rearrange("p a b c -> p (a b c)")

            # ---------------- pre = int @ Wc1top  (folded-T) ----------------
            pre = big.tile([128, NBT, NK, 128], BF16)
            pre_flat = pre[:].rearrange("p a b c -> p (a b c)")
            NBLK = NBT * NK  # 160 column blocks of 128
            for g in range(NBLK // 4):
                ps = PS[:, (g % 2) * 512:(g % 2) * 512 + 512]
                c0 = g * 512
                for half in range(2):
                    rows = slice(half * D, (half + 1) * D)
                    nc.tensor.matmul(
                        ps[rows, :], wtop[rows, :], int_flat[rows, c0:c0 + 512],
                        start=True, stop=True)
                nc.vector.tensor_copy(pre_flat[:, c0:c0 + 512], ps[:])

            # PSUM layout (f32 cols):
            #   [0:1536)    PE-arg region A (banks 0-2)
            #   [1536:3072) PE-arg region B (banks 3-5); also hW lives at
            #               [2048:3072) when DVE-arg bts exist (disjoint in
            #               time from region B usage if PE_ARG small)
            #   [3072:3392) cp (128, 320)
            #   [3584:4096) 4 rotating psN slots (128, 128)
            # GRU phase reuses [0:3072).
            cp_ps = PS[:, 3072:3072 + NBT * ND]
            NSLOT = 4

            # ---------------- time loop ----------------
            for t in range(T):
                h = htiles[t % 2]
                hn_out = htiles[(t + 1) % 2]

                # hW for DVE-arg bts: (128, 1024) at [2048:3072)
                dve_bts = [bt for bt in range(NBT) if bt not in PE_ARG]
                if dve_bts:
                    psH = PS[:, 2048:2048 + 1024]
                    hw_sb = sm.tile([128, 8 * 128], BF16, tag="hw")
                    for i in range(0, len(dve_bts), 4):
                        bts = dve_bts[i:i + 4]
                        for j, bt in enumerate(bts):
                            nc.tensor.matmul(
                                psH[:, (i + j) * 128:(i + j + 1) * 128],
                                wbot[:], h[0:D, bt * 128:(bt + 1) * 128],
                                start=True, stop=True)
                        w = len(bts) * 128
                        nc.vector.tensor_copy(
                            hw_sb[:, i * 128:i * 128 + w],
                            psH[:, i * 128:i * 128 + w])

                x1s = {}
                for bt in range(NBT):
                    x1 = x1p.tile([128, NK, 128], BF16, tag="x1")
                    x1s[bt] = x1
                    if bt in PE_ARG:
                        base = 0 if (bt % 2 == 0) else 1536
                        ps = PS[:, base:base + NK * 128]
                        for s0 in range(0, NK * 128, 512):
                            w = min(512, NK * 128 - s0)
                            c0 = bt * NK * 128
                            nc.tensor.matmul(ps[:, s0:s0 + w], identb[:],
                                             pre_flat[:, c0 + s0:c0 + s0 + w],
                                             start=True, stop=False)
                        for s0 in range(0, NK * 128, 512):
                            w = min(512, NK * 128 - s0)
                            nk = w // 128
                            hrep = h[0:D, bt * 128:(bt + 1) * 128].unsqueeze(1)
                            hrep = hrep.broadcast_to([D, nk, 128])
                            nc.tensor.matmul(
                                ps[:, s0:s0 + w].rearrange(
                                    "p (k b) -> p k b", k=nk),
                                wbot[:], hrep, start=False, stop=True)
                        nc.scalar.activation(
                            x1[:],
                            ps[:].rearrange("p (k b) -> p k b", k=NK),
                            AF.Tanh, bias=bc1f[:], scale=1.0)
                    else:
                        di = dve_bts.index(bt)
                        arg = x1p.tile([128, NK, 128], BF16, tag="arg")
                        hv = hw_sb[:, di * 128:(di + 1) * 128]
                        hv = hv.unsqueeze(1).broadcast_to([128, NK, 128])
                        nc.vector.tensor_tensor(
                            arg[:], pre[:, bt, :, :], hv, ALU.add)
                        nc.scalar.activation(
                            x1[:], arg[:], AF.Tanh, bias=bc1f[:], scale=1.0)
                    # cp for this bt
                    for k in range(NK):
                        col = bt * ND + 2 * k
                        nc.tensor.matmul(cp_ps[:, col:col + 2],
                                         x1[:, k, :], wc2p[:],
                                         start=True, stop=True)

                # softmax + weighted sum, per bt-quarter for exp/alpha and
                # per bt for prod/fold
                wslot = 0
                for q in range(NQ):
                    bts = range(q * 4, (q + 1) * 4)
                    e_q = smq.tile([128, 4, ND], BF16, tag="e")
                    nc.scalar.activation(
                        e_q[:],
                        cp_ps[:, q * 4 * ND:(q + 1) * 4 * ND].rearrange(
                            "p (a n) -> p a n", n=ND),
                        AF.Exp)
                    s_q = smq.tile([128, 4], F32, tag="s")
                    nc.vector.tensor_reduce(s_q[:], e_q[:], axis=AX.X,
                                            op=ALU.add)
                    r_q = smq.tile([128, 4], F32, tag="r")
                    nc.vector.reciprocal(r_q[:], s_q[:])
                    al_q = smq.tile([128, 4, ND], BF16, tag="al")
                    nc.vector.tensor_tensor(
                        al_q[:], e_q[:],
                        r_q[:].unsqueeze(2).broadcast_to([128, 4, ND]),
                        ALU.mult)
                    for j, bt in enumerate(bts):
                        prod = prp.tile([128, D, ND], BF16, tag="prod")
                        av = al_q[:, j, :].unsqueeze(1)
                        av = av.broadcast_to([128, D, ND])
                        nc.vector.tensor_tensor(
                            prod[:], intB[:, bt, :, :], av, ALU.mult)
                        # fold: 20 transpose-matmuls accumulated in one slot
                        psn = PS[0:D, 3584 + (wslot % NSLOT) * 128:
                                 3584 + (wslot % NSLOT) * 128 + 128]
                        wslot += 1
                        for n in range(ND):
                            nc.tensor.matmul(
                                psn[:], prod[:, :, n], identb[:],
                                start=(n == 0), stop=(n == ND - 1))
                        nc.scalar.activation(
                            embrhs[0:D, bt * 128:(bt + 1) * 128],
                            psn[:], AF.Identity)

                # --- emb per quarter: psE slots (16, 512) in banks 4-5
                embT = sm.tile([DE, BL], BF16, tag="embT")
                for q in range(NQ):
                    qs = slice(q * 512, (q + 1) * 512)
                    psE = PS[0:DE, 2048 + (q % 2) * 512:2048 + (q % 2) * 512 + 512]
                    nc.tensor.matmul(psE[:], weX[:], embrhs[:, qs],
                                     start=True, stop=True)
                    nc.vector.tensor_copy(embT[:, qs], psE[:])

                # --- GRU gates in two b-halves
                HB = BL // 2
                for hb in range(2):
                    cs = slice(hb * HB, (hb + 1) * HB)
                    ps_r = PS[0:D, 0:1024]
                    ps_z = PS[0:D, 1024:2048]
                    ps_i = PS[0:D, 2048:3072]
                    ps_h = PS[64:128, 2048:3072]
                    for q in range(2):
                        qs = slice(hb * HB + q * 512, hb * HB + (q + 1) * 512)
                        qo = slice(q * 512, (q + 1) * 512)
                        nc.tensor.matmul(ps_r[:, qo], wih_rz[:, 0:D],
                                         embT[:, qs], start=True, stop=False)
                        nc.tensor.matmul(ps_r[:, qo], whh_rz[:, 0:D],
                                         h[0:D, qs], start=False, stop=True)
                        nc.tensor.matmul(ps_z[:, qo], wih_rz[:, D:128],
                                         embT[:, qs], start=True, stop=False)
                        nc.tensor.matmul(ps_z[:, qo], whh_rz[:, D:128],
                                         h[0:D, qs], start=False, stop=True)
                        nc.tensor.matmul(ps_i[:, qo], wih_n[:],
                                         embT[:, qs], start=True, stop=True)
                        nc.tensor.matmul(ps_h[:, qo], whh_n[:],
                                         h[0:D, qs], start=True, stop=True)
                    # r,z = sigmoid via tanh trick
                    tr = gp.tile([D, HB], BF16, tag="gA")
                    nc.scalar.activation(tr[:], ps_r[:], AF.Tanh,
                                         bias=brz[0:D, :], scale=0.5)
                    r_sb = gp.tile([D, HB], BF16, tag="gB")
                    nc.vector.tensor_scalar(r_sb[:], tr[:], 0.5, 0.5,
                                            ALU.mult, ALU.add)
                    tz = gp.tile([D, HB], BF16, tag="gC")
                    nc.scalar.activation(tz[:], ps_z[:], AF.Tanh,
                                         bias=brz[D:128, :], scale=0.5)
                    z_sb = gp.tile([D, HB], BF16, tag="gD")
                    nc.vector.tensor_scalar(z_sb[:], tz[:], 0.5, 0.5,
                                            ALU.mult, ALU.add)
                    # rhn = (hn + bhh_n) * r ; narg = (inn + bih_n) + rhn
                    rhn = gp.tile([D, HB], BF16, tag="gE")
                    nc.vector.scalar_tensor_tensor(
                        rhn[:], ps_h[:], bhhn[:], r_sb[:], ALU.add, ALU.mult)
                    narg = gp.tile([D, HB], BF16, tag="gF")
                    nc.vector.scalar_tensor_tensor(
                        narg[:], ps_i[:], bihn[:], rhn[:], ALU.add, ALU.add)
                    n_sb = gp.tile([D, HB], BF16, tag="gG")
                    nc.scalar.activation(n_sb[:], narg[:], AF.Tanh)
                    # h_new = n + z*(h-n)
                    hmn = gp.tile([D, HB], BF16, tag="gB2")
                    nc.vector.tensor_tensor(hmn[:], h[0:D, cs], n_sb[:],
                                            ALU.subtract)
                    zt = gp.tile([D, HB], BF16, tag="gA2")
                    nc.vector.tensor_tensor(zt[:], hmn[:], z_sb[:], ALU.mult)
                    nc.vector.tensor_tensor(hn_out[0:D, cs], zt[:], n_sb[:],
                                            ALU.add)

                # --- y = h_new @ [Wo; bo] : psY = PS[0:2, 0:2048)
                for q in range(4):
                    psY = PS[0:2, 1024 + (q % 2) * 512:1024 + (q % 2) * 512 + 512]
                    qs = slice(q * 512, (q + 1) * 512)
                    nc.tensor.matmul(psY[:], wo65[:], hn_out[:, qs],
                                     start=True, stop=True)
                    nc.vector.tensor_copy(embrhs[64:66, qs], psY[:])
                nc.sync.dma_start(out_d[t, :, :].rearrange("b c -> c b"),
                                  embrhs[64:66, :])

    return nc


def _prep_inputs(last_x_rel, zo, intermediate, We, be, Wih, Whh, bih, bhh,
                 Wc1, bc1, Wc2, bc2, Wo, bo):
    """Shard + pack host-side.  Returns list of per-core input dicts."""
    lx = np.asarray(last_x_rel, np.float32)
    zo = np.asarray(zo, np.float32)
    inter = np.asarray(intermediate, np.float32)
    We = np.asarray(We, np.float32)
    be = np.asarray(be, np.float32)
    Wih = np.asarray(Wih, np.float32)
    Whh = np.asarray(Whh, np.float32)
    bih = np.asarray(bih, np.float32)
    bhh = np.asarray(bhh, np.float32)
    Wc1 = np.asarray(Wc1, np.float32)
    bc1 = np.asarray(bc1, np.float32)
    Wc2 = np.asarray(Wc2, np.float32)
    bc2 = np.asarray(bc2, np.float32)
    Wo = np.asarray(Wo, np.float32)
    bo = np.asarray(bo, np.float32)

    identb = np.eye(128, dtype=np.float32).astype(BF)
    wtop_dup = np.concatenate([Wc1[0:D], Wc1[0:D]], 0).astype(BF)
    wbot_wide = np.concatenate([Wc1[D:2 * D], Wc1[D:2 * D]], 1).astype(BF)
    bc1_fold = np.concatenate([bc1, bc1])[:, None].astype(np.float32)
    wc2_pair = np.zeros((128, 2), np.float32)
    wc2_pair[0:D, 0] = Wc2[:, 0]
    wc2_pair[D:128, 1] = Wc2[:, 0]
    wc2_pair = wc2_pair.astype(BF)
    weX = np.concatenate([We, be[None, :]], 0).astype(BF)   # (67, 16)
    wih_rz = Wih[:, 0:128].astype(BF)
    whh_rz = Whh[:, 0:128].astype(BF)
    wih_n = Wih[:, 128:192].astype(BF)
    whh_n = Whh[:, 128:192].astype(BF)
    brz_half = (0.5 * (bih[0:128] + bhh[0:128]))[:, None].astype(np.float32)
    bih_n = bih[128:192][:, None].astype(np.float32)
    bhh_n = bhh[128:192][:, None].astype(np.float32)
    wo65 = np.concatenate([Wo, bo[None, :]], 0).astype(BF)  # (65, 2)

    iv = inter.reshape(ND, NC, NBT, 128, D)
    zv = zo[-1]  # (B, D)
    in_maps = []
    for c in range(NC):
        int_nat = np.ascontiguousarray(
            iv[:, c].transpose(2, 1, 0, 3).reshape(128, NBT * ND * D)
        ).astype(BF)
        # b-major: (128b, (bt, d, n))
        intB_nat = np.ascontiguousarray(
            iv[:, c].transpose(2, 1, 3, 0).reshape(128, NBT * D * ND)
        ).astype(BF)
        h0T = np.ascontiguousarray(zv[c * BL:(c + 1) * BL].T).astype(BF)
        y0T = np.concatenate([lx[c * BL:(c + 1) * BL].T,
                              np.ones((1, BL), np.float32)], 0).astype(BF)
        in_maps.append({
            "int_nat": int_nat, "intB_nat": intB_nat, "h0T": h0T, "y0T": y0T,
            "identb": identb,
            "wtop_dup": wtop_dup, "wbot_wide": wbot_wide,
            "bc1_fold": bc1_fold, "wc2_pair": wc2_pair,
            "weX": weX, "wih_rz": wih_rz, "whh_rz": whh_rz,
            "wih_n": wih_n, "whh_n": whh_n, "brz_half": brz_half,
            "bih_n": bih_n, "bhh_n": bhh_n, "wo65": wo65,
        })
    return in_maps


def kernel(pred_len, last_x_rel, zo, zg, intermediate, We, be, Wih, Whh,
           bih, bhh, Wc1, bc1, Wc2, bc2, Wo, bo, _trace=False):
    assert int(pred_len) == T, f"kernel compiled for pred_len={T}"
    from concourse.bass_utils import run_bass_kernel_spmd

    if "nc" not in _BUILT:
        _BUILT["nc"] = _build()
    nc = _BUILT["nc"]

    in_maps = _prep_inputs(last_x_rel, zo, intermediate, We, be, Wih, Whh,
                           bih, bhh, Wc1, bc1, Wc2, bc2, Wo, bo)
    res = run_bass_kernel_spmd(nc, in_maps, list(range(NC)), trace=_trace)
    ys = np.concatenate([res.results[c]["ys"].astype(np.float32)
                         for c in range(NC)], axis=1)
    if _trace:
        _BUILT["last_result"] = res
    return ys
